# revision 2
# baseline (speedup 1.0000x reference)
"""Differential attention kernel for Trainium2, 8-core SPMD.

Problem (hardcoded shapes): B=2, S=2048, D=2048, H=16 heads, head_dim=128,
dual-chunk q/k dim 64.  out = (softmax(q1k1*s+m) - lam*softmax(q2k2*s+m)) @ v,
then output projection.

Sharding: batch x head-group.  Core c handles batch c//4 and heads
(c%4)*4 .. +4.  Each core computes its 4 heads' QKV columns (tensor
parallel on c_attn output cols), full attention for those heads, and a
partial output projection (tensor parallel on c_proj input rows).  The
4 partial projections per batch are summed on host (the "unshard").

All matmuls run as float32r (TF32-like fast fp32 mode, 1 cyc/row at
moving-dim >= 256).

Layout strategy (everything "transposed" so contraction dims land on
SBUF partitions, no on-chip transposes anywhere):
  - host passes X^T [D, S] per batch
  - QKV phase produces, SBUF-resident: per head h a packed Q^T tile
    q12[h] [128, S] (rows 0:64 = q1^T, 64:128 = q2^T), ditto k12[h],
    and V in natural layout v_h[h] [128, KC, 128] (partition = s%128).
  - scores computed transposed per head/q-tile: s^T [k, q] psum tiles,
    with the two dual-chunk scores row-packed into PE row groups 0-1 /
    2-3 (K=64 each) so they run concurrently;
  - exp on ScalarE (scale folded in; attn mask bias folded in when
    nontrivial) at [128, 1024] granularity straight out of PSUM;
  - softmax denominators via ones-vector matmuls (partition-dim
    reduction on the PE), E2's chain deferred until E1's finishes so
    score psum tiles can double-buffer inside 8 banks;
  - PV gives O^T [hd, q] which is exactly the lhsT layout the output
    projection wants; 1/denominator applied via partition-broadcast
    then fused multiply-subtract on the VectorE.
"""

import numpy as np

import concourse.bass as bass
import concourse.mybir as mybir
import concourse.tile as tile
from concourse import bacc
from concourse.bass_utils import run_bass_kernel_spmd

F32 = mybir.dt.float32
F32R = mybir.dt.float32r
EXP = mybir.ActivationFunctionType.Exp
MULT = mybir.AluOpType.mult

B, S, D, H = 2, 2048, 2048, 16
HD = D // H            # 128 full head dim
QD = HD // 2           # 64 dual-chunk q/k dim
N_CORES = 8
HPC = H // (N_CORES // B)   # 4 heads per core
CPB = N_CORES // B          # 4 cores per batch
SCALE = float(HD) ** -0.5
KC = S // 128          # 16 k-chunks
DC = D // 128          # 16 d-chunks
SCW = 512              # QKV s-chunk width
XSUB = 4               # d-chunks per xt subtile


def build_program(lam: float, mask_trivial: bool):
    nc = bacc.Bacc("TRN2", target_bir_lowering=False, debug=False,
                   enable_asserts=False, num_devices=N_CORES)

    xt = nc.dram_tensor("xt", [D, S], F32, kind="ExternalInput").ap()
    wqk = nc.dram_tensor("wqk", [D, 2 * HPC * HD], F32, kind="ExternalInput").ap()
    wv = nc.dram_tensor("wv", [D, HPC * HD], F32, kind="ExternalInput").ap()
    wp = nc.dram_tensor("wp", [HPC * HD, D], F32, kind="ExternalInput").ap()
    maskb = nc.dram_tensor("maskb", [KC, 128], F32, kind="ExternalInput").ap()
    onescol = nc.dram_tensor("onescol", [128, 1], F32, kind="ExternalInput").ap()
    y = nc.dram_tensor("y", [S, D], F32, kind="ExternalOutput").ap()

    NQK = 2 * HPC  # 8 qk c-tiles of 128
    with tile.TileContext(nc) as tc:
        with (
            tc.tile_pool(name="consts", bufs=1) as cpool,
            tc.tile_pool(name="qkvres", bufs=1) as qkvpool,
        ):
            ones_t = cpool.tile([128, 1], F32R, tag="ones")
            nc.sync.dma_start(ones_t[:], onescol.bitcast(F32R))
            mask_t = cpool.tile([128, KC], F32, tag="mask")
            nc.sync.dma_start(mask_t[:], maskb.rearrange("c p -> p c"))

            q12, k12, v_h = [], [], []
            for h in range(HPC):
                q12.append(qkvpool.tile([128, S], F32R, tag=f"q{h}",
                                        name=f"q12_{h}"))
                k12.append(qkvpool.tile([128, S], F32R, tag=f"k{h}",
                                        name=f"k12_{h}"))
                v_h.append(qkvpool.tile([128, KC, HD], F32R, tag=f"v{h}",
                                        name=f"v_{h}"))

            # ---------------- Phase 1: QKV projections ----------------
            with (
                tc.tile_pool(name="xtp", bufs=6) as xtp,
                tc.tile_pool(name="wqkp", bufs=3) as wqkp,
                tc.tile_pool(name="wvp", bufs=1) as wvp,
                tc.tile_pool(name="qkv_ev", bufs=3) as evp,
                tc.tile_pool(name="qkv_ps", bufs=4, space="PSUM") as psp,
            ):
                wv_t = wvp.tile([128, DC, HPC * HD], F32R, tag="wv")
                ct_order = [4, 0, 5, 1, 6, 2, 7, 3]
                w_cache: dict = {}

                def load_w(ct):
                    w = wqkp.tile([128, DC, 128], F32R, tag="wqk",
                                  name=f"wqk_{ct}")
                    nc.sync.dma_start(
                        w[:],
                        wqk[:, ct * 128:(ct + 1) * 128]
                        .rearrange("(c p) m -> p c m", p=128).bitcast(F32R))
                    return w

                for sc in range(S // SCW):
                    ss = slice(sc * SCW, (sc + 1) * SCW)
                    xt_sub = []
                    for i in range(DC // XSUB):
                        t = xtp.tile([128, XSUB, SCW], F32R, tag="xt",
                                     name=f"xt_{sc}_{i}")
                        nc.sync.dma_start(
                            t[:],
                            xt[i * XSUB * 128:(i + 1) * XSUB * 128, ss]
                            .rearrange("(c p) s -> p c s", p=128).bitcast(F32R))
                        xt_sub.append(t)
                        if sc == 0:
                            # interleave the wv slices with the xt subtiles
                            # so the first V matmul chain starts early
                            sl = slice(i * XSUB, (i + 1) * XSUB)
                            nc.sync.dma_start(
                                wv_t[:, sl, :],
                                wv.rearrange("(c p) n -> p c n", p=128)[:, sl, :]
                                .bitcast(F32R))

                    def xtc(dc):
                        return xt_sub[dc // XSUB][:, dc % XSUB, :]

                    # snake the c-tile order so pool-resident weight tiles
                    # from the previous s-chunk get reused at the boundary
                    order = ct_order if sc % 2 == 0 else ct_order[::-1]

                    # V (natural layout): lhsT = X^T chunk, rhs = Wv
                    for st in range(SCW // 128):
                        ps = psp.tile([128, 512], F32, tag="ps")
                        for dc in range(DC):
                            nc.tensor.matmul(
                                ps[:],
                                xtc(dc)[:, st * 128:(st + 1) * 128],
                                wv_t[:, dc, :],
                                start=(dc == 0), stop=(dc == DC - 1))
                        stg = sc * (SCW // 128) + st
                        for h in range(HPC):
                            nc.vector.tensor_copy(
                                v_h[h][:, stg, :],
                                ps[:, h * HD:(h + 1) * HD])

                    # Q^T / K^T c-tiles (k first so attention unblocks early)
                    for ct in order:
                        w_t = w_cache.pop(ct, None)
                        if w_t is None:
                            w_t = load_w(ct)
                        ps = psp.tile([128, 512], F32, tag="ps")
                        for dc in range(DC):
                            nc.tensor.matmul(
                                ps[:], w_t[:, dc, :], xtc(dc),
                                start=(dc == 0), stop=(dc == DC - 1))
                        dst = k12[ct - HPC] if ct >= HPC else q12[ct]
                        nc.vector.tensor_copy(dst[:, ss], ps[:])
                        last_w = (ct, w_t)
                    # only the most recent tile survives the pool rotation
                    w_cache = {last_w[0]: last_w[1]}

            # ---------------- Phase 2: differential attention ----------------
            gpool_cm = tc.tile_pool(name="gbuf", bufs=1)
            gpool = gpool_cm.__enter__()
            g_tiles = []
            for h in range(HPC):
                g_tiles.append(
                    gpool.tile([HD, S], F32R, tag=f"g{h}", name=f"g{h}"))
            wpp_cm = tc.tile_pool(name="wpp", bufs=1)
            wpp = wpp_cm.__enter__()
            wp_tiles: list = []

            def load_wp(h):
                w = wpp.tile([HD, D], F32R, tag=f"wp{h}", name=f"wp{h}")
                nc.sync.dma_start(
                    w[:], wp[h * HD:(h + 1) * HD, :].bitcast(F32R))
                wp_tiles.append(w)

            with (
                tc.tile_pool(name="e1p", bufs=3) as e1pool,
                tc.tile_pool(name="e2p", bufs=8) as e2pool,
                tc.tile_pool(name="rp", bufs=2) as rpool,
                tc.tile_pool(name="Rp", bufs=2) as Rpool,
                tc.tile_pool(name="tp", bufs=1) as tpool,
                tc.tile_pool(name="att_s", bufs=2, space="PSUM") as spsum,
                tc.tile_pool(name="att_o", bufs=2, space="PSUM") as opsum,
                tc.tile_pool(name="att_d", bufs=2, space="PSUM") as dpsum,
            ):
                for h in range(HPC):
                    if h == HPC - 1:
                        # DMA is idle during attention; stage the first
                        # projection weights before the phase boundary
                        load_wp(0)
                        load_wp(1)
                    for qt in range(S // 512):
                        qs = slice(qt * 512, (qt + 1) * 512)
                        o1 = opsum.tile([128, 512], F32, tag="o")
                        d1 = dpsum.tile([1, 512], F32, tag="d")
                        e2_tiles = []
                        e1_pending = None  # software pipeline: scores/exp of
                        # pair pg are emitted before denom/PV of pair pg-1 so
                        # the PE never has to sit out an exp

                        def emit_dpv1(pg, e1):
                            for j in range(2):
                                kc = pg * 2 + j
                                js = slice(j * 512, (j + 1) * 512)
                                first = (kc == 0)
                                last = (kc == KC - 1)
                                nc.tensor.matmul(d1[:], ones_t[:], e1[:, js],
                                                 start=first, stop=last)
                                nc.tensor.matmul(o1[:], v_h[h][:, kc, :],
                                                 e1[:, js],
                                                 start=first, stop=last)

                        for pg in range(KC // 2):
                            s1p = spsum.tile([128, 1024], F32, tag="s")
                            s2p = spsum.tile([128, 1024], F32, tag="s")
                            for j in range(2):
                                kc = pg * 2 + j
                                js = slice(j * 512, (j + 1) * 512)
                                nc.tensor.matmul(
                                    s1p[:, js],
                                    k12[h][0:QD, kc * 128:(kc + 1) * 128],
                                    q12[h][0:QD, qs], start=True, stop=True)
                                nc.tensor.matmul(
                                    s2p[:, js],
                                    k12[h][QD:128, kc * 128:(kc + 1) * 128],
                                    q12[h][QD:128, qs], start=True, stop=True)
                            e1 = e1pool.tile([128, 1024], F32R, tag="e1")
                            e2 = e2pool.tile([128, 1024], F32R, tag="e2")
                            if mask_trivial:
                                nc.scalar.activation(e1[:], s1p[:], EXP,
                                                     scale=SCALE)
                                nc.scalar.activation(e2[:], s2p[:], EXP,
                                                     scale=SCALE)
                            else:
                                for j in range(2):
                                    kc = pg * 2 + j
                                    js = slice(j * 512, (j + 1) * 512)
                                    mb = mask_t[:, kc:kc + 1]
                                    nc.scalar.activation(e1[:, js], s1p[:, js],
                                                         EXP, bias=mb,
                                                         scale=SCALE)
                                    nc.scalar.activation(e2[:, js], s2p[:, js],
                                                         EXP, bias=mb,
                                                         scale=SCALE)
                            e2_tiles.append(e2)
                            if e1_pending is not None:
                                emit_dpv1(pg - 1, e1_pending)
                            e1_pending = e1
                        emit_dpv1(KC // 2 - 1, e1_pending)

                        # evacuate E1 results, then second softmax chain
                        r1 = rpool.tile([1, 512], F32, tag="r")
                        nc.vector.reciprocal(r1[:], d1[:])

                        o2 = opsum.tile([128, 512], F32, tag="o")
                        d2 = dpsum.tile([1, 512], F32, tag="d")
                        for kc in range(KC):
                            e2 = e2_tiles[kc // 2]
                            js = slice((kc % 2) * 512, (kc % 2 + 1) * 512)
                            first = (kc == 0)
                            last = (kc == KC - 1)
                            nc.tensor.matmul(d2[:], ones_t[:], e2[:, js],
                                             start=first, stop=last)
                            nc.tensor.matmul(o2[:], v_h[h][:, kc, :],
                                             e2[:, js],
                                             start=first, stop=last)
                        r2 = rpool.tile([1, 512], F32, tag="r")
                        nc.vector.reciprocal(r2[:], d2[:])

                        R1 = Rpool.tile([128, 512], F32, tag="R")
                        nc.gpsimd.partition_broadcast(R1[:], r1[:])
                        R2 = Rpool.tile([128, 512], F32, tag="R")
                        nc.gpsimd.partition_broadcast(R2[:], r2[:])

                        u1 = tpool.tile([128, 512], F32, tag="u1")
                        nc.vector.tensor_mul(u1[:], o1[:], R1[:])
                        u2 = tpool.tile([128, 512], F32, tag="u2")
                        nc.vector.scalar_tensor_tensor(
                            u2[:], o2[:], lam, R2[:], op0=MULT, op1=MULT)
                        nc.vector.tensor_sub(g_tiles[h][:, qs], u1[:], u2[:])

            # ---------------- Phase 3: output projection ----------------
            with (
                tc.tile_pool(name="wpp2", bufs=1) as wpp2,
                tc.tile_pool(name="yev", bufs=3) as yev,
                tc.tile_pool(name="proj_ps", bufs=4, space="PSUM") as ppsum,
            ):
                for h in (2, 3):
                    w = wpp2.tile([HD, D], F32R, tag=f"wp{h}", name=f"wp{h}")
                    nc.sync.dma_start(
                        w[:], wp[h * HD:(h + 1) * HD, :].bitcast(F32R))
                    wp_tiles.append(w)
                for st in range(S // 128):
                    yt = yev.tile([128, D], F32, tag="yt")
                    for et in range(D // 512):
                        ps = ppsum.tile([128, 512], F32, tag="ps")
                        for h in range(HPC):
                            nc.tensor.matmul(
                                ps[:],
                                g_tiles[h][:, st * 128:(st + 1) * 128],
                                wp_tiles[h][:, et * 512:(et + 1) * 512],
                                start=(h == 0), stop=(h == HPC - 1))
                        nc.vector.tensor_copy(yt[:, et * 512:(et + 1) * 512],
                                              ps[:])
                    nc.sync.dma_start(y[st * 128:(st + 1) * 128, :], yt[:])
            wpp_cm.__exit__(None, None, None)
            gpool_cm.__exit__(None, None, None)

    nc.compile()
    return nc


_PROGRAM_CACHE: dict = {}


def _get_program(lam: float, mask_trivial: bool):
    key = (round(lam, 9), mask_trivial)
    if key not in _PROGRAM_CACHE:
        _PROGRAM_CACHE[key] = build_program(lam, mask_trivial)
    return _PROGRAM_CACHE[key]


def make_in_maps(hidden_states, attention_mask, W_attn, b_attn, W_proj):
    in_maps = []
    for c in range(N_CORES):
        b = c // CPB
        h0 = (c % CPB) * HPC
        xt = np.ascontiguousarray(hidden_states[b].T)
        cols = []
        for h in range(h0, h0 + HPC):
            cols.append(W_attn[:, h * QD:(h + 1) * QD])              # q1
            cols.append(W_attn[:, D // 2 + h * QD:D // 2 + (h + 1) * QD])  # q2
        for h in range(h0, h0 + HPC):
            cols.append(W_attn[:, D + h * QD:D + (h + 1) * QD])      # k1
            cols.append(W_attn[:, D + D // 2 + h * QD:D + D // 2 + (h + 1) * QD])
        wqk = np.ascontiguousarray(np.concatenate(cols, axis=1))
        wv = np.ascontiguousarray(W_attn[:, 2 * D + h0 * HD:2 * D + (h0 + HPC) * HD])
        wpm = np.ascontiguousarray(W_proj[h0 * HD:(h0 + HPC) * HD, :])
        maskb = np.ascontiguousarray(
            ((1.0 - attention_mask[b]) * -10000.0).reshape(KC, 128)
        ).astype(np.float32)
        in_maps.append({
            "xt": xt.astype(np.float32),
            "wqk": wqk.astype(np.float32),
            "wv": wv.astype(np.float32),
            "wp": wpm.astype(np.float32),
            "maskb": maskb,
            "onescol": np.ones((128, 1), np.float32),
        })
    return in_maps


def kernel(hidden_states, attention_mask, W_attn, b_attn, W_proj, b_proj,
           lambda_param, _trace=False):
    hidden_states = np.asarray(hidden_states, np.float32)
    attention_mask = np.asarray(attention_mask, np.float32)
    W_attn = np.asarray(W_attn, np.float32)
    b_attn = np.asarray(b_attn, np.float32)
    W_proj = np.asarray(W_proj, np.float32)
    b_proj = np.asarray(b_proj, np.float32)
    lam = float(np.asarray(lambda_param))

    if np.any(b_attn != 0.0):
        raise NotImplementedError("nonzero b_attn not supported")

    mask_trivial = bool(np.all(attention_mask == 1.0))
    nc = _get_program(lam, mask_trivial)
    in_maps = make_in_maps(hidden_states, attention_mask, W_attn, b_attn,
                           W_proj)
    try:
        res = run_bass_kernel_spmd(nc, in_maps, core_ids=list(range(N_CORES)),
                                   trace=_trace)
    except ModuleNotFoundError:
        res = run_bass_kernel_spmd(nc, in_maps, core_ids=list(range(N_CORES)),
                                   trace=False)

    out = np.empty((B, S, D), np.float32)
    for b in range(B):
        acc = res.results[b * CPB]["y"].astype(np.float32).copy()
        for c in range(b * CPB + 1, (b + 1) * CPB):
            acc += res.results[c]["y"]
        out[b] = acc + b_proj[None, :]
    kernel.last_exec_time_ns = res.exec_time_ns
    if res.instructions_and_trace is not None:
        kernel.last_trace_path = res.instructions_and_trace[1]
    return out


kernel.last_exec_time_ns = None
kernel.last_trace_path = None

